# revision 1
# baseline (speedup 1.0000x reference)
"""GPT2 attention, head-sharded across 8 NeuronCores.

Strategy (per sharding_hint): tensor-parallel over heads. 16 heads / 8 cores
= 2 heads per core. w_attn columns are split in the 3 (key|query|value)
groups by head, each core computes its heads' qkv projection + attention,
and the per-core contexts are concatenated (all-gather) on the host.
"""
import numpy as np
import jax
import jax.numpy as jnp

NUM_HEADS = 16
HIDDEN = 2048
HEAD = HIDDEN // NUM_HEADS  # 128
B, S = 2, 2048
NC = 8
HPC = NUM_HEADS // NC  # heads per core = 2
SCALE = 1.0 / np.sqrt(HEAD).astype(np.float32)


def _shard_step(enc, mask, w_loc, b_loc):
    # enc: [B,S,HIDDEN]; w_loc: [HIDDEN, 3*HPC*HEAD]; b_loc: [3*HPC*HEAD]
    qkv = enc @ w_loc + b_loc                      # [B,S,3*HPC*HEAD]
    k, q, v = jnp.split(qkv, 3, axis=-1)           # each [B,S,HPC*HEAD]

    def to_heads(x):
        return x.reshape(B, S, HPC, HEAD).transpose(0, 2, 1, 3)  # [B,hpc,S,c]

    q, k, v = to_heads(q), to_heads(k), to_heads(v)
    scores = jnp.einsum('bhfc,bhtc->bhft', q, k) * SCALE
    scores = scores * mask                          # multiplicative, post-scale
    attn = jax.nn.softmax(scores, axis=-1)
    ctx = jnp.einsum('bhft,bhtc->bhfc', attn, v)    # [B,hpc,S,c]
    return ctx.transpose(0, 2, 1, 3).reshape(B, S, HPC * HEAD)


def _split_weights(w_attn, b_attn):
    # columns: [0:H]=key, [H:2H]=query, [2H:3H]=value; head h -> h*HEAD:(h+1)*HEAD
    w_shards, b_shards = [], []
    for d in range(NC):
        cols = []
        for g in range(3):  # key, query, value groups
            base = g * HIDDEN + d * HPC * HEAD
            cols.append(np.arange(base, base + HPC * HEAD))
        idx = np.concatenate(cols)
        w_shards.append(np.asarray(w_attn)[:, idx])
        b_shards.append(np.asarray(b_attn)[idx])
    return np.stack(w_shards), np.stack(b_shards)


_pmapped = None


def kernel(encodings, attention_masks, w_attn, b_attn):
    global _pmapped
    enc = np.asarray(encodings, dtype=np.float32)
    mask = np.asarray(attention_masks, dtype=np.float32)[0, 0]  # [S,S]
    w_sh, b_sh = _split_weights(w_attn, b_attn)

    try:
        devs = jax.devices()
        if len(devs) >= NC:
            if _pmapped is None:
                _pmapped = jax.pmap(_shard_step,
                                    in_axes=(None, None, 0, 0),
                                    devices=devs[:NC])
            ctx = _pmapped(jnp.asarray(enc), jnp.asarray(mask),
                           jnp.asarray(w_sh), jnp.asarray(b_sh))
            ctx = np.asarray(ctx)                   # [NC,B,S,HPC*HEAD]
        else:
            raise RuntimeError("need 8 devices")
    except Exception:
        ctx = np.stack([np.asarray(_shard_step(jnp.asarray(enc),
                                               jnp.asarray(mask),
                                               jnp.asarray(w_sh[d]),
                                               jnp.asarray(b_sh[d])))
                        for d in range(NC)])

    # gather: device d holds heads [d*HPC, (d+1)*HPC) -> concat on head axis
    out = ctx.reshape(NC, B, S, HPC, HEAD).transpose(1, 2, 0, 3, 4)
    return np.ascontiguousarray(out.reshape(B, S, HIDDEN), dtype=np.float32)
